# revision 49
# baseline (speedup 1.0000x reference)
"""Trainium2 Bass kernel for MoD (mixture-of-depths) routing FFN.

Semantics (matching the reference):
  w = x @ W_r + b_r                        # [B, S] router weights
  t_b = K-th largest of w[b, :]            # per-row threshold (K=512)
  selected: w > t_b (strict; ties at threshold dropped)
  out[b, s] = w[b,s] * (gelu(x[b,s] @ W1 + b1) @ W2 + b2)   if selected
  out[b, s] = x[b, s]                                        otherwise

Sharding: 8 cores; cores (2b, 2b+1) form a pair handling batch row b.
Each core routes half the row, ranks are computed from an AllGather'ed
router-weight vector (exact selection via "count of w_j >= w_i <= K-1",
counting split across DVE/GpSimd/ACT), the selected tokens (max K-1 per
row) are compacted into K slots via matmul-based stream compaction, and
the FFN runs tensor-parallel over the pair (W1 column-split / W2
row-split, bf16 compute) with pair AllReduces of the partial outputs.
Routing, selection and the output residual path stay fully fp32.
"""

from contextlib import ExitStack

import numpy as np

import concourse.bass as bass
import concourse.tile as tile
from concourse import mybir
from concourse.bass import IndirectOffsetOnAxis
from concourse.bass_utils import run_bass_kernel_spmd
from concourse.masks import make_identity
from concourse.tile_rust import add_dep_helper

F32 = mybir.dt.float32
BF16 = mybir.dt.bfloat16
I32 = mybir.dt.int32

NC_CORES = 8
DEBUG_DUMPS = False

# rank-count column split across engines (out of S/128 columns)
# (GpSimd/Pool cannot run TensorScalar-with-accumulate, so DVE + ACT only)
RANK_DVE_FRAC = 0.5
RANK_GPS_FRAC = 0.0


def build_mod_kernel(nc, S, D, DFF, K):
    """Emit the per-core SPMD program. Pair = (2b, 2b+1) handles row b.

    Inputs (per-core):
      x_own  [S/2, D] f32   this core's half-row (router + residual src)
      x_row  [S, D]   f32   the full row (gather source for the FFN)
      wr     [1, D]   f32   router weight
      br     [1, 1]   f32   router bias
      w1     [D, DFFH] bf16  W1 column shard
      w2     [DFFH, D] bf16  W2 row shard
      b1s    [1, DFFH] f32  b1 shard
      b2h    [1, D]   bf16  0.5 * b2
      hoff   [1, 1]   f32   h * S/2  (0 for even cores, S/2 for odd)
    Output:
      out    [S/2, D] f32
    """
    HALF = S // 2
    DFFH = DFF // 2
    CAP = K                      # slots per row (max selected = K-1 < CAP)
    KT = HALF // 128             # x tiles per core (16)
    TT = S // 128                # token tiles per row (32)
    NJ = CAP // 128              # slot tiles (4)
    ND = D // 128                # d 128-tiles (16)
    NM = DFFH // 128             # dff-col tiles (32)
    NDC = D // 512               # d 512-chunks (4)
    NG = 1                       # d-chunks per mm2 psum group
    NGRP = NDC // NG             # mm2 groups == number of split AllReduces

    x_own = nc.declare_dram_parameter("x_own", [HALF, D], F32, isOutput=False)
    x_row = nc.declare_dram_parameter("x_row", [S, D], F32, isOutput=False)
    wr = nc.declare_dram_parameter("wr", [1, D], F32, isOutput=False)
    br = nc.declare_dram_parameter("br", [1, 1], F32, isOutput=False)
    w1 = nc.declare_dram_parameter("w1", [D, DFFH], BF16, isOutput=False)
    w2 = nc.declare_dram_parameter("w2", [DFFH, D], BF16, isOutput=False)
    b1s = nc.declare_dram_parameter("b1s", [1, DFFH], F32, isOutput=False)
    b2h = nc.declare_dram_parameter("b2h", [1, D], BF16, isOutput=False)
    hoff = nc.declare_dram_parameter("hoff", [1, 1], F32, isOutput=False)
    out = nc.declare_dram_parameter("out", [HALF, D], F32, isOutput=True)

    # Internal DRAM for collectives (pair groups).
    ag_in = nc.dram_tensor("ag_in", [1, HALF], F32)
    ag_out = nc.dram_tensor("ag_out", [2, HALF], F32)
    # partial FFN outputs, chunk-major so each AllReduce slice is contiguous
    ar_in = nc.dram_tensor("ar_in", [NGRP, CAP, NG * 512], F32)
    ar_out = nc.dram_tensor("ar_out", [NGRP, CAP, NG * 512], F32)
    pairs = [[2 * b, 2 * b + 1] for b in range(NC_CORES // 2)]

    with tile.TileContext(nc) as tc, ExitStack() as ctx:
        pc = ctx.enter_context(tc.tile_pool(name="const", bufs=1))
        pr = ctx.enter_context(tc.tile_pool(name="route", bufs=1))

        # ---- constants ----
        ident = pc.tile([128, 128], F32, name="ident")
        make_identity(nc, ident[:])
        ones128 = pc.tile([128, 1], F32, name="ones128")
        nc.vector.memset(ones128[:], 1.0)
        ones1b = pc.tile([1, 128], BF16, name="ones1b")
        nc.vector.memset(ones1b[:], 1.0)
        # U strict-upper triangulars (as stored): U[q, p] = 1 iff q < p
        uTT = pc.tile([TT, TT], F32, name="uTT")
        nc.gpsimd.memset(uTT[:], 0.0)
        nc.gpsimd.affine_select(
            out=uTT[:], in_=uTT[:], compare_op=mybir.AluOpType.is_ge,
            fill=1.0, base=0, pattern=[[-1, TT]], channel_multiplier=1,
        )
        u128 = pc.tile([128, 128], F32, name="u128")
        nc.gpsimd.memset(u128[:], 0.0)
        nc.gpsimd.affine_select(
            out=u128[:], in_=u128[:], compare_op=mybir.AluOpType.is_ge,
            fill=1.0, base=0, pattern=[[-1, 128]], channel_multiplier=1,
        )
        s_iota = pc.tile([128, CAP], F32, name="s_iota")
        nc.gpsimd.iota(s_iota[:], pattern=[[1, CAP]], base=0,
                       channel_multiplier=0, allow_small_or_imprecise_dtypes=True)
        # compact lhsT rows, bf16-exact: [p+1, c, gate] per token column c
        tg3 = pc.tile([128, 3 * TT], BF16, name="tg3")
        tg3v = tg3[:].rearrange("p (c three) -> p c three", three=3)
        nc.gpsimd.iota(tg3v[:, :, 0], pattern=[[0, TT]], base=1,
                       channel_multiplier=1, allow_small_or_imprecise_dtypes=True)
        nc.gpsimd.iota(tg3v[:, :, 1], pattern=[[1, TT]], base=0,
                       channel_multiplier=0, allow_small_or_imprecise_dtypes=True)

        # ---- small input broadcasts ----
        wr1 = pc.tile([1, D], F32, name="wr1")
        nc.sync.dma_start(wr1[:], wr.ap())
        wr_bc = pc.tile([128, D], F32, name="wr_bc")
        nc.gpsimd.partition_broadcast(wr_bc[:], wr1[:], 128)
        br1 = pc.tile([1, 1], F32, name="br1")
        nc.sync.dma_start(br1[:], br.ap())
        br_bc = pc.tile([128, 1], F32, name="br_bc")
        nc.gpsimd.partition_broadcast(br_bc[:], br1[:], 128)
        ho1 = pc.tile([1, 1], F32, name="ho1")
        nc.sync.dma_start(ho1[:], hoff.ap())
        ho_bc = pc.tile([128, 1], F32, name="ho_bc")
        nc.gpsimd.partition_broadcast(ho_bc[:], ho1[:], 128)
        # b1_sb[p, m] = b1s[m*128 + p]
        b1_sb = pc.tile([128, NM], F32, name="b1_sb")
        nc.sync.dma_start(b1_sb[:], b1s.ap().rearrange("o (m p) -> (o p) m", p=128))
        b2_sb = pc.tile([1, D], BF16, name="b2_sb")
        nc.sync.dma_start(b2_sb[:], b2h.ap())

        # ---- phase R: router dot (residual copy deferred to FFN window) ----
        w_mine = pr.tile([128, KT], F32, name="w_mine")
        with tc.tile_pool(name="xs", bufs=6) as px, \
             tc.tile_pool(name="jr", bufs=1) as pjr:
            for k in range(KT):
                xt = px.tile([128, D], F32)
                nc.sync.dma_start(xt[:], x_own.ap()[k * 128:(k + 1) * 128, :])
                jt = pjr.tile([128, D], F32, tag="jR")
                nc.vector.scalar_tensor_tensor(
                    out=jt[:], in0=xt[:], scalar=1.0, in1=wr_bc[:],
                    op0=mybir.AluOpType.bypass, op1=mybir.AluOpType.mult,
                    accum_out=w_mine[:, k:k + 1],
                )
            w_full = pr.tile([128, KT], F32, name="w_full")
            nc.vector.tensor_scalar_add(w_full[:], w_mine[:], br_bc[:, 0:1])
            # DRAM layout: element l = k*128 + p
            nc.sync.dma_start(
                ag_in.ap().rearrange("o (k p) -> (o p) k", p=128), w_full[:])

        # ---- AllGather router weights within pair ----
        ag_cc = nc.gpsimd.collective_compute(
            "AllGather", mybir.AluOpType.bypass, replica_groups=pairs,
            ins=[ag_in.ap()], outs=[ag_out.ap()],
        )

        # ---- phase RANK: exact top-K selection ----
        # counts[t] = #{j: w_j >= w_t}, decomposed into 4 half-passes
        # (targets: own/other columns via mask, compare set: own/other
        # half-row).  The (own columns x own half) pass needs no collective,
        # so it overlaps the AllGather.  Masked-out columns compare against
        # BIGV and contribute exactly 0 (DVE) / -HALF (ACT sign-sum, a
        # uniform offset fixed in the final conversion).  Columns split
        # across DVE and ACT.
        BIGV = 1e30
        # own-half compare set, available pre-AllGather from ag_in
        wro = pr.tile([1, HALF], F32, name="wro")
        nc.sync.dma_start(wro[:], ag_in.ap())
        w_bco = pr.tile([128, HALF], F32, name="w_bco")
        nc.gpsimd.partition_broadcast(w_bco[:], wro[:], 128)
        # own token values duplicated into both column halves, then masked
        w_dup = pr.tile([128, TT], F32, name="w_dup")
        for h2 in range(2):
            nc.sync.dma_start(
                w_dup[:, h2 * KT:(h2 + 1) * KT],
                ag_in.ap().rearrange("o (k p) -> (o p) k", p=128))
        # ownmask[p, c] = 1 iff c in [16h, 16h+16)
        c_iota = pr.tile([128, TT], F32, name="c_iota")
        nc.gpsimd.iota(c_iota[:], pattern=[[1, TT]], base=0,
                       channel_multiplier=0, allow_small_or_imprecise_dtypes=True)
        lo_bc = pr.tile([128, 1], F32, name="lo_bc")
        nc.vector.tensor_scalar_mul(lo_bc[:], ho_bc[:], 1.0 / 128.0)
        m_ge = pr.tile([128, TT], F32, name="m_ge")
        nc.vector.tensor_scalar(out=m_ge[:], in0=c_iota[:], scalar1=lo_bc[:, 0:1],
                                scalar2=None, op0=mybir.AluOpType.is_ge)
        hi_bc = pr.tile([128, 1], F32, name="hi_bc")
        nc.vector.tensor_scalar_add(hi_bc[:], lo_bc[:], float(KT))
        m_lt = pr.tile([128, TT], F32, name="m_lt")
        nc.vector.tensor_scalar(out=m_lt[:], in0=c_iota[:], scalar1=hi_bc[:, 0:1],
                                scalar2=None, op0=mybir.AluOpType.is_lt)
        ownm = pr.tile([128, TT], F32, name="ownm")
        nc.vector.tensor_tensor(out=ownm[:], in0=m_ge[:], in1=m_lt[:],
                                op=mybir.AluOpType.mult)
        invm = pr.tile([128, TT], F32, name="invm")
        nc.vector.tensor_scalar(out=invm[:], in0=ownm[:], scalar1=-1.0,
                                scalar2=1.0, op0=mybir.AluOpType.mult,
                                op1=mybir.AluOpType.add)

        def masked(dst, mask, maskinv_big, vals):
            # dst = mask ? vals : BIGV  ==  mask*vals + maskinv*BIGV (exact)
            nc.vector.tensor_tensor(out=dst[:], in0=mask[:], in1=vals[:],
                                    op=mybir.AluOpType.mult)
            nc.vector.tensor_tensor(out=dst[:], in0=dst[:], in1=maskinv_big[:],
                                    op=mybir.AluOpType.add)

        big_inv = pr.tile([128, TT], F32, name="big_inv")
        nc.vector.tensor_scalar_mul(big_inv[:], invm[:], BIGV)
        big_own = pr.tile([128, TT], F32, name="big_own")
        nc.vector.tensor_scalar_mul(big_own[:], ownm[:], BIGV)
        wt_pre = pr.tile([128, TT], F32, name="wt_pre")
        masked(wt_pre, ownm, big_inv, w_dup)
        nwt_pre = pr.tile([128, TT], F32, name="nwt_pre")
        nc.vector.tensor_scalar_mul(nwt_pre[:], wt_pre[:], -1.0)

        n_dve = max(1, int(round(TT * RANK_DVE_FRAC)))

        def half_pass(cdst, craw_dst, wset, wt, nwt, pjd, pja, tag):
            for c in range(n_dve):
                jt = pjd.tile([128, HALF], F32, tag=f"jD{tag}")
                nc.vector.tensor_scalar(
                    out=jt[:], in0=wset[:], scalar1=wt[:, c:c + 1],
                    scalar2=None, op0=mybir.AluOpType.is_ge,
                    op1=mybir.AluOpType.add, accum_out=cdst[:, c:c + 1],
                )
            for c in range(n_dve, TT):
                jt = pja.tile([128, HALF], F32, tag=f"jA{tag}")
                nc.scalar.activation(
                    out=jt[:], in_=wset[:],
                    func=mybir.ActivationFunctionType.Sign,
                    bias=nwt[:, c:c + 1], scale=1.0,
                    accum_out=craw_dst[:, c:c + 1],
                )

        cps_t = [pr.tile([128, TT], F32, name=f"cnt{i}") for i in range(4)]
        crw_t = [pr.tile([128, TT], F32, name=f"crw{i}") for i in range(4)]
        if DEBUG_DUMPS:
            for t_ in cps_t + crw_t:
                nc.vector.memset(t_[:], 0.0)
        with tc.tile_pool(name="jkd", bufs=1) as pjd, \
             tc.tile_pool(name="jka", bufs=1) as pja:
            # pre-AllGather pass: own columns x own half
            half_pass(cps_t[0], crw_t[0], w_bco, wt_pre, nwt_pre, pjd, pja, "0")
            # post-AllGather tiles
            w_tok = pr.tile([128, TT], F32, name="w_tok")
            for h2 in range(2):
                nc.sync.dma_start(
                    w_tok[:, h2 * KT:(h2 + 1) * KT],
                    ag_out.ap()[h2:h2 + 1, :].rearrange("o (k p) -> (o p) k",
                                                        p=128))
            wt_post = pr.tile([128, TT], F32, name="wt_post")
            masked(wt_post, invm, big_own, w_tok)
            nwt_post = pr.tile([128, TT], F32, name="nwt_post")
            nc.vector.tensor_scalar_mul(nwt_post[:], wt_post[:], -1.0)
            # other-half compare set: row (1 - h) of ag_out (runtime row id)
            oth_f = pr.tile([128, 1], F32, name="oth_f")
            nc.vector.tensor_scalar(
                out=oth_f[:], in0=ho_bc[:], scalar1=-1.0 / HALF, scalar2=1.0,
                op0=mybir.AluOpType.mult, op1=mybir.AluOpType.add)
            oth_i = pr.tile([128, 1], I32, name="oth_i")
            nc.vector.tensor_copy(oth_i[:], oth_f[:])
            w_bcx = pr.tile([128, HALF], F32, name="w_bcx")
            nc.gpsimd.indirect_dma_start(
                out=w_bcx[:], out_offset=None, in_=ag_out.ap(),
                in_offset=IndirectOffsetOnAxis(ap=oth_i[:, 0:1], axis=0),
            )
            half_pass(cps_t[1], crw_t[1], w_bcx, wt_pre, nwt_pre, pjd, pja, "1")
            half_pass(cps_t[2], crw_t[2], w_bco, wt_post, nwt_post, pjd, pja, "2")
            half_pass(cps_t[3], crw_t[3], w_bcx, wt_post, nwt_post, pjd, pja, "3")
        counts = pr.tile([128, TT], F32, name="counts")
        if n_dve > 0:
            nc.vector.tensor_tensor(out=cps_t[0][:, 0:n_dve],
                                    in0=cps_t[0][:, 0:n_dve],
                                    in1=cps_t[1][:, 0:n_dve],
                                    op=mybir.AluOpType.add)
            nc.vector.tensor_tensor(out=cps_t[2][:, 0:n_dve],
                                    in0=cps_t[2][:, 0:n_dve],
                                    in1=cps_t[3][:, 0:n_dve],
                                    op=mybir.AluOpType.add)
            nc.vector.tensor_tensor(out=counts[:, 0:n_dve],
                                    in0=cps_t[0][:, 0:n_dve],
                                    in1=cps_t[2][:, 0:n_dve],
                                    op=mybir.AluOpType.add)
        if n_dve < TT:
            c0 = n_dve
            nc.vector.tensor_tensor(out=crw_t[0][:, c0:TT],
                                    in0=crw_t[0][:, c0:TT],
                                    in1=crw_t[1][:, c0:TT],
                                    op=mybir.AluOpType.add)
            nc.vector.tensor_tensor(out=crw_t[2][:, c0:TT],
                                    in0=crw_t[2][:, c0:TT],
                                    in1=crw_t[3][:, c0:TT],
                                    op=mybir.AluOpType.add)
            nc.vector.tensor_tensor(out=crw_t[0][:, c0:TT],
                                    in0=crw_t[0][:, c0:TT],
                                    in1=crw_t[2][:, c0:TT],
                                    op=mybir.AluOpType.add)
            # craw_total = ssum_true - S;  count_ge = (ssum + S + 1) / 2
            nc.vector.tensor_scalar(
                out=counts[:, c0:TT], in0=crw_t[0][:, c0:TT],
                scalar1=float(2 * S + 1), scalar2=0.5,
                op0=mybir.AluOpType.add, op1=mybir.AluOpType.mult)

        # selected  <=>  #{j: w_j >= w_i} <= K-1  (ties at the K-th value drop)
        sel = pr.tile([128, TT], F32, name="sel")
        nc.vector.tensor_scalar(out=sel[:], in0=counts[:], scalar1=float(K - 1),
                                scalar2=None, op0=mybir.AluOpType.is_le)
        unsel = pr.tile([128, TT], F32, name="unsel")
        nc.vector.tensor_scalar(out=unsel[:], in0=counts[:], scalar1=float(K - 1),
                                scalar2=None, op0=mybir.AluOpType.is_gt)
        gate = pr.tile([128, TT], F32, name="gate")
        nc.vector.tensor_tensor(out=gate[:], in0=sel[:], in1=w_tok[:],
                                op=mybir.AluOpType.mult)
        nc.vector.tensor_copy(tg3v[:, :, 2], gate[:])

        # ---- phase PREFIX: exclusive prefix-sum of sel over t = c*128+p ----
        with tc.tile_pool(name="pps", bufs=1, space="PSUM") as pps:
            colT_ps = pps.tile([TT, 1], F32, name="colT_ps")
            nc.tensor.matmul(colT_ps[:], lhsT=sel[:], rhs=ones128[:],
                             start=True, stop=True)
            colT = pr.tile([TT, 1], F32, name="colT")
            nc.vector.tensor_copy(colT[:], colT_ps[:])
            pos_ps = pps.tile([128, TT], F32, name="pos_ps")
            nc.tensor.matmul(pos_ps[:], lhsT=colT[:].to_broadcast([TT, 128]),
                             rhs=uTT[:], start=True, stop=False)
            nc.tensor.matmul(pos_ps[:], lhsT=u128[:], rhs=sel[:],
                             start=False, stop=True)
            pos = pr.tile([128, TT], F32, name="pos")
            nc.vector.tensor_copy(pos[:], pos_ps[:])
        pos_m = pr.tile([128, TT], F32, name="pos_m")
        nc.vector.scalar_tensor_tensor(
            out=pos_m[:], in0=unsel[:], scalar=float(4 * CAP + 7), in1=pos[:],
            op0=mybir.AluOpType.mult, op1=mybir.AluOpType.add,
        )

        # ---- phase COMPACT: slot -> (p+1, c, gate) via bf16 matmuls ----
        tok_i = []   # int32 gather offsets per slot tile
        gate_s = []  # f32 per-slot gates
        dest_i = []  # int32 scatter offsets (OOB for pad/other-half)
        with tc.tile_pool(name="pcm", bufs=1, space="PSUM") as pcm, \
             tc.tile_pool(name="pmm", bufs=3) as pmm, \
             tc.tile_pool(name="ptp", bufs=2, space="PSUM") as ptp:
            cps = pcm.tile([3, CAP], F32, name="cps")
            for c in range(TT):
                mt = pmm.tile([128, CAP], BF16, tag="mt")
                nc.vector.tensor_scalar(
                    out=mt[:], in0=s_iota[:], scalar1=pos_m[:, c:c + 1],
                    scalar2=None, op0=mybir.AluOpType.is_equal,
                )
                nc.tensor.matmul(cps[:], lhsT=tg3[:, 3 * c:3 * c + 3], rhs=mt[:],
                                 start=(c == 0), stop=(c == TT - 1))
            compact = pr.tile([3, CAP], F32, name="compact")
            nc.vector.tensor_copy(compact[:], cps[:])
            for j in range(NJ):
                tp = ptp.tile([128, 3], F32, tag="tp")
                nc.tensor.transpose(tp[:], compact[:, j * 128:(j + 1) * 128],
                                    ident[0:3, 0:3])
                cpj = pr.tile([128, 3], F32, name=f"cpj{j}")
                nc.vector.tensor_copy(cpj[:], tp[:])
                gate_s.append(cpj)
                # tokp1 = 128*c + (p+1)  == token id + 1; 0 for pad slots
                tokp1 = pr.tile([128, 1], F32, name=f"tokp1{j}")
                nc.vector.scalar_tensor_tensor(
                    out=tokp1[:], in0=cpj[:, 1:2], scalar=128.0, in1=cpj[:, 0:1],
                    op0=mybir.AluOpType.mult, op1=mybir.AluOpType.add)
                # gather offset: max(tokp1 - 1, 0) -> int
                tif = pr.tile([128, 1], F32, name=f"tif{j}")
                nc.vector.tensor_scalar(
                    out=tif[:], in0=tokp1[:], scalar1=-1.0, scalar2=0.0,
                    op0=mybir.AluOpType.add, op1=mybir.AluOpType.max,
                )
                tii = pr.tile([128, 1], I32, name=f"tii{j}")
                nc.vector.tensor_copy(tii[:], tif[:])
                tok_i.append(tii)
                # scatter offset: (tokp1 - 1) - hoff, OOB for pad/other-half
                df = pr.tile([128, 1], F32, name=f"df{j}")
                nc.vector.scalar_tensor_tensor(
                    out=df[:], in0=tokp1[:], scalar=-1.0, in1=ho_bc[:],
                    op0=mybir.AluOpType.add, op1=mybir.AluOpType.subtract,
                )
                ok1 = pr.tile([128, 1], F32, name=f"ok1{j}")
                nc.vector.tensor_scalar(out=ok1[:], in0=df[:], scalar1=0.0,
                                        scalar2=None, op0=mybir.AluOpType.is_ge)
                ok2 = pr.tile([128, 1], F32, name=f"ok2{j}")
                nc.vector.tensor_scalar(out=ok2[:], in0=df[:],
                                        scalar1=float(HALF - 1), scalar2=None,
                                        op0=mybir.AluOpType.is_le)
                okm = pr.tile([128, 1], F32, name=f"okm{j}")
                nc.vector.tensor_tensor(out=okm[:], in0=ok1[:], in1=ok2[:],
                                        op=mybir.AluOpType.mult)
                # dfm = okm * (df - BIG) + BIG  (df when ok, BIG when not)
                BIG = float(8 * HALF + 11)
                dfs = pr.tile([128, 1], F32, name=f"dfs{j}")
                nc.vector.tensor_scalar_add(dfs[:], df[:], -BIG)
                dfm = pr.tile([128, 1], F32, name=f"dfm{j}")
                nc.vector.scalar_tensor_tensor(
                    out=dfm[:], in0=okm[:], scalar=BIG, in1=dfs[:],
                    op0=mybir.AluOpType.bypass, op1=mybir.AluOpType.mult)
                nc.vector.tensor_scalar_add(dfm[:], dfm[:], BIG)
                dii = pr.tile([128, 1], I32, name=f"dii{j}")
                nc.vector.tensor_copy(dii[:], dfm[:])
                dest_i.append(dii)

        # ---- phase GATHER: xg rows -> transpose -> xgT (bf16) ----
        if DEBUG_DUMPS:
            dbg_compact = nc.dram_tensor("dbg_compact", [3, CAP], F32)
            nc.sync.dma_start(dbg_compact.ap(), compact[:])
            dbg_route = nc.dram_tensor("dbg_route", [128, 5 * TT], F32)
            nc.sync.dma_start(dbg_route.ap()[:, 0 * TT:1 * TT], w_tok[:])
            nc.sync.dma_start(dbg_route.ap()[:, 1 * TT:2 * TT], counts[:])
            nc.sync.dma_start(dbg_route.ap()[:, 2 * TT:3 * TT], sel[:])
            nc.sync.dma_start(dbg_route.ap()[:, 3 * TT:4 * TT], pos[:])
            nc.sync.dma_start(dbg_route.ap()[:, 4 * TT:5 * TT], pos_m[:])
            dbg_rank = nc.dram_tensor("dbg_rank", [128, 8 * TT], F32)
            for idx, t_ in enumerate([w_dup, ownm, wt_pre, wt_post] + cps_t):
                nc.sync.dma_start(dbg_rank.ap()[:, idx * TT:(idx + 1) * TT],
                                  t_[:])
            dbg_xg = nc.dram_tensor("dbg_xg", [128, D], F32)
            dbg_xgT = nc.dram_tensor("dbg_xgT", [128, ND * CAP], F32)
            dbg_h = nc.dram_tensor("dbg_h", [128, NM * CAP], F32)
        xgT = pr.tile([128, ND, CAP], BF16, name="xgT")
        last_gather = None
        with tc.tile_pool(name="pxg", bufs=2) as pxg, \
             tc.tile_pool(name="ptg", bufs=3, space="PSUM") as ptg:
            for j in range(NJ):
                xg = pxg.tile([128, D], F32, tag="xg")
                last_gather = nc.gpsimd.indirect_dma_start(
                    out=xg[:], out_offset=None, in_=x_row.ap(),
                    in_offset=IndirectOffsetOnAxis(ap=tok_i[j][:, 0:1], axis=0),
                )
                if DEBUG_DUMPS and j == 0:
                    nc.sync.dma_start(dbg_xg.ap(), xg[:])
                for k in range(ND):
                    tps = ptg.tile([128, 128], F32, tag="tps")
                    nc.tensor.transpose(tps[:], xg[:, k * 128:(k + 1) * 128],
                                        ident[:])
                    nc.vector.tensor_copy(
                        xgT[:, k, j * 128:(j + 1) * 128], tps[:])
        if DEBUG_DUMPS:
            xgT_f = pr.tile([128, ND * CAP], F32, name="xgT_f")
            nc.vector.tensor_copy(xgT_f[:], xgT[:].rearrange("p a b -> p (a b)"))
            nc.sync.dma_start(dbg_xgT.ap(), xgT_f[:])

        # ---- deferred residual copy: out = x (DRAM->DRAM) ----
        # Anchored after the xg gathers so it streams on the (then idle)
        # sync ring during mm1; the final scatter overwrites selected rows
        # afterwards (explicit dep added at the scatter site).
        residual_dmas = []
        for k in range(KT // 4):
            r = nc.sync.dma_start(
                out.ap()[k * 512:(k + 1) * 512, :],
                x_own.ap()[k * 512:(k + 1) * 512, :])
            add_dep_helper(r.ins, last_gather.ins, sync=True,
                           reason="residual copy after routing window")
            residual_dmas.append(r)

        # ---- phase MM1 + gelu: h[dffcol, toks] = gelu(xg @ W1 + b1) ----
        h_all = pr.tile([128, NM, CAP], BF16, name="h_all")
        MG = min(4, NM)  # m-tiles per W1 stream chunk
        with tc.tile_pool(name="pw1", bufs=8) as pw1, \
             tc.tile_pool(name="ph1", bufs=2, space="PSUM") as ph1:
            for mg in range(NM // MG):
                hps = [ph1.tile([128, CAP], F32, tag=f"hp{i}", name=f"hp{i}")
                       for i in range(MG)]
                for k in range(ND):
                    w1c = pw1.tile([128, MG * 128], BF16, tag="w1c")
                    nc.scalar.dma_start(
                        w1c[:],
                        w1.ap()[k * 128:(k + 1) * 128,
                                mg * MG * 128:(mg + 1) * MG * 128])
                    for i in range(MG):
                        nc.tensor.matmul(
                            hps[i][:], lhsT=w1c[:, i * 128:(i + 1) * 128],
                            rhs=xgT[:, k, :], start=(k == 0), stop=(k == ND - 1))
                for i in range(MG):
                    m = mg * MG + i
                    nc.scalar.activation(
                        out=h_all[:, m, :], in_=hps[i][:],
                        func=mybir.ActivationFunctionType.Gelu_apprx_tanh,
                        bias=b1_sb[:, m:m + 1], scale=1.0)

        if DEBUG_DUMPS:
            h_f = pr.tile([128, NM * CAP], F32, name="h_f")
            nc.vector.tensor_copy(h_f[:], h_all[:].rearrange("p a b -> p (a b)"))
            nc.sync.dma_start(dbg_h.ap(), h_f[:])

        # ---- phase MM2: blk[toks, d] = h.T @ W2 + 0.5*b2, then AllReduce ----
        with tc.tile_pool(name="pw2", bufs=3) as pw2, \
             tc.tile_pool(name="pb2", bufs=2, space="PSUM") as pb2, \
             tc.tile_pool(name="pbs", bufs=3) as pbs:
            for g in range(NGRP):
                bps = [pb2.tile([128, 512], F32, tag=f"bp{i}", name=f"bp{i}")
                       for i in range(NG * NJ)]
                for m in range(NM):
                    w2c = pw2.tile([128, NG * 512], BF16, tag="w2c")
                    nc.scalar.dma_start(
                        w2c[:],
                        w2.ap()[m * 128:(m + 1) * 128,
                                NG * g * 512:(NG * g + NG) * 512])
                    for j in range(NJ):
                        for i in range(NG):
                            nc.tensor.matmul(
                                bps[NG * j + i][:],
                                lhsT=h_all[:, m, j * 128:(j + 1) * 128],
                                rhs=w2c[:, i * 512:(i + 1) * 512],
                                start=(m == 0), stop=False)
                for j in range(NJ):
                    for i in range(NG):
                        n = NG * g + i
                        nc.tensor.matmul(
                            bps[NG * j + i][:], lhsT=ones1b[:],
                            rhs=b2_sb[:, n * 512:(n + 1) * 512],
                            start=False, stop=True)
                        bsb = pbs.tile([128, 512], F32, tag="bsb")
                        nc.vector.tensor_copy(bsb[:], bps[NG * j + i][:])
                        nc.sync.dma_start(
                            ar_in.ap()[g, j * 128:(j + 1) * 128,
                                       i * 512:(i + 1) * 512], bsb[:])
                # AllReduce this chunk while the next one computes
                nc.gpsimd.collective_compute(
                    "AllReduce", mybir.AluOpType.add, replica_groups=pairs,
                    ins=[ar_in.ap()[g]], outs=[ar_out.ap()[g]],
                )

        # ---- phase FINAL: gate * ar -> scatter into out (residual holds) ----
        # per-(g, j) column slices so chunk g's combine overlaps chunk g+1's
        # matmuls and AllReduce
        CW = NG * 512
        with tc.tile_pool(name="pfa", bufs=3) as pfa:
            for g in range(NGRP):
                for j in range(NJ):
                    art = pfa.tile([128, CW], F32, tag="art")
                    nc.sync.dma_start(
                        art[:], ar_out.ap()[g, j * 128:(j + 1) * 128, :])
                    nc.vector.tensor_scalar(
                        out=art[:], in0=art[:], scalar1=gate_s[j][:, 2:3],
                        scalar2=None, op0=mybir.AluOpType.mult)
                    sc = nc.gpsimd.indirect_dma_start(
                        out=out.ap(),
                        out_offset=IndirectOffsetOnAxis(
                            ap=dest_i[j][:, 0:1], axis=0),
                        in_=art[:], in_offset=None,
                        element_offset=g * CW,
                        bounds_check=HALF - 1, oob_is_err=False,
                    )
                    for r in residual_dmas:
                        add_dep_helper(sc.ins, r.ins, sync=True,
                                       reason="scatter after residual copy")
    return nc


# ---------------------------------------------------------------------------
# Host-side wrapper
# ---------------------------------------------------------------------------

_BUILT = {}


def _get_nc(S, D, DFF, K):
    key = (S, D, DFF, K)
    if key not in _BUILT:
        from concourse import bacc
        nc = bacc.Bacc(trn_type="TRN2", num_devices=NC_CORES, debug=False)
        build_mod_kernel(nc, S, D, DFF, K)
        nc.compile()
        _BUILT[key] = nc
    return _BUILT[key]


def make_in_maps(x, W_r, b_r, W1, b1, W2, b2, S, D, DFF, K):
    import ml_dtypes
    bf = ml_dtypes.bfloat16
    HALF = S // 2
    DFFH = DFF // 2
    in_maps = []
    w1sh, w2sh, b1sh = [], [], []
    for h in range(2):
        w1sh.append(np.ascontiguousarray(
            W1[:, h * DFFH:(h + 1) * DFFH]).astype(bf))
        w2sh.append(np.ascontiguousarray(
            W2[h * DFFH:(h + 1) * DFFH, :]).astype(bf))
        b1sh.append(np.ascontiguousarray(b1[h * DFFH:(h + 1) * DFFH]).reshape(1, DFFH))
    b2half = (0.5 * b2).astype(bf).reshape(1, D)
    for c in range(NC_CORES):
        b, h = c // 2, c % 2
        in_maps.append({
            "x_own": np.ascontiguousarray(x[b, h * HALF:(h + 1) * HALF, :]),
            "x_row": np.ascontiguousarray(x[b]),
            "wr": W_r.reshape(1, D).astype(np.float32),
            "br": b_r.reshape(1, 1).astype(np.float32),
            "w1": w1sh[h],
            "w2": w2sh[h],
            "b1s": b1sh[h].astype(np.float32),
            "b2h": b2half,
            "hoff": np.array([[h * HALF]], dtype=np.float32),
        })
    return in_maps


def kernel(x, W_r, b_r, W1, b1, W2, b2, position_ids=None, cache_position=None,
           **unused):
    x = np.asarray(x, dtype=np.float32)
    W_r = np.asarray(W_r, dtype=np.float32)
    b_r = np.asarray(b_r, dtype=np.float32)
    W1 = np.asarray(W1, dtype=np.float32)
    b1 = np.asarray(b1, dtype=np.float32)
    W2 = np.asarray(W2, dtype=np.float32)
    b2 = np.asarray(b2, dtype=np.float32)
    B, S, D = x.shape
    DFF = W1.shape[1]
    K = 512
    HALF = S // 2
    nc = _get_nc(S, D, DFF, K)
    in_maps = make_in_maps(x, W_r, b_r, W1, b1, W2, b2, S, D, DFF, K)
    res = run_bass_kernel_spmd(nc, in_maps, list(range(NC_CORES)))
    out = np.empty((B, S, D), dtype=np.float32)
    for c in range(NC_CORES):
        b, h = c // 2, c % 2
        out[b, h * HALF:(h + 1) * HALF, :] = res.results[c]["out"]
    return out
